# revision 13
# baseline (speedup 1.0000x reference)
"""GAT (2-layer, PyG-style) distributed Bass kernel for 8 Trainium2 NeuronCores.

Strategy (graph/data parallel; halo exchange done at input-sharding time):
  - Nodes are partitioned into 8 contiguous blocks; core c owns destination
    nodes [c*N/8, (c+1)*N/8) and all edges incident to them.
  - The host shards the inputs per core as edge-ordered, transposed bf16
    feature arrays: xdT[:, i] = x[src_i], xddT[:, i] = x[dst_i] for each
    edge slot i (dst-tile-major layout, 128-slot chunks).  This is the halo
    exchange of neighbor features performed eagerly during input
    distribution, so the device sees only contiguous DMA.
  - Device per dst tile (128 nodes, CS chunks of 128 edges):
      * per chunk: matmul (xdT_chunk)^T @ [W|W@a_src] -> [xh|es] in PSUM;
        es+ed accumulated in one PSUM bank via two matmuls
        (lhsT=xdT chunk then lhsT=xddT chunk);
      * batched leaky-relu + exp on the scalar engine per quarter-tile;
      * one vector multiply per quarter forms [att*xh | ea] rows (bf16);
      * per chunk one bf16 selection matmul (seT built by a single batched
        is_equal per tile) segment-reduces [msg|ea] into the accumulator;
      * self loops are handled as an extra slot per tile (identity
        selection -> added in the epilogue, no matmul).
  - Between layers the host reassembles h and builds the layer-2 dup arrays
    the same way.
"""

import math
import os
import sys

for _p in ("/opt/trn_rl_repo", "/root/.axon_site/_ro/trn_rl_repo"):
    if os.path.isdir(_p) and _p not in sys.path:
        sys.path.insert(0, _p)

import numpy as np
import ml_dtypes
from contextlib import ExitStack

import concourse.bacc as bacc
import concourse.bass as bass
import concourse.tile as tile
from concourse import mybir
from concourse.bass_utils import run_bass_kernel_spmd

F32 = mybir.dt.float32
BF16 = mybir.dt.bfloat16
AF = mybir.ActivationFunctionType
ALU = mybir.AluOpType
BF = ml_dtypes.bfloat16

NEG_SLOPE = 0.2
P = 128
PAD_DST = 200.0  # sentinel dst_local for pad/self slots


def _round_up(a, b):
    return (a + b - 1) // b * b


# --------------------------------------------------------------------------
# host-side graph preprocessing (pure indexing, no float math on features)
# --------------------------------------------------------------------------

class EdgeLayout:
    def __init__(self, src, dst, N, n_cores):
        self.N = N
        self.n_cores = n_cores
        assert N % n_cores == 0
        self.npc = N // n_cores
        self.T = math.ceil(self.npc / P)

        src = src.astype(np.int64)
        dst = dst.astype(np.int64)

        per_core = []
        max_cnt = 0
        for c in range(n_cores):
            lo = c * self.npc
            sel = (dst >= lo) & (dst < lo + self.npc)
            s_c = src[sel]
            d_c = dst[sel]
            dl = d_c - lo
            t_all = dl >> 7
            order = np.argsort(t_all, kind="stable")
            s_c, d_c, dl, t_all = s_c[order], d_c[order], dl[order], t_all[order]
            cnt = np.bincount(t_all, minlength=self.T)
            max_cnt = max(max_cnt, int(cnt.max()))
            per_core.append((s_c, d_c, dl, cnt))

        self.CS = max(1, math.ceil(max_cnt / P))   # chunks per tile
        CS = self.CS
        self.nslots = self.T * CS * P

        self.src_slots = []   # int64 [nslots], -1 for pad
        self.dst_slots = []
        self.dl_slots = []    # float32 [nslots], PAD_DST for pad
        for c in range(n_cores):
            s_c, d_c, dl, cnt = per_core[c]
            ss = np.full(self.nslots, -1, np.int64)
            ds = np.full(self.nslots, -1, np.int64)
            dd = np.full(self.nslots, PAD_DST, np.float32)
            starts = np.concatenate([[0], np.cumsum(cnt)])[:-1]
            pos_in_tile = np.arange(len(s_c)) - starts[dl >> 7]
            pos = (dl >> 7) * (CS * P) + pos_in_tile
            ss[pos] = s_c
            ds[pos] = d_c
            dd[pos] = (dl & 127).astype(np.float32)
            self.src_slots.append(ss)
            self.dst_slots.append(ds)
            self.dl_slots.append(dd)

    def dup_T(self, tbl_bf, slots):
        """tbl_bf: [N, C] bf16 -> [C, nslots] bf16 (zeros at pad slots)."""
        out = np.zeros((self.nslots, tbl_bf.shape[1]), BF)
        real = slots >= 0
        out[real] = tbl_bf[slots[real]]
        return np.ascontiguousarray(out.T)

    def dlt(self, c):
        """[128, T*CS] bf16: dl of slot (t, k, p) at [p, t*CS+k]."""
        dd = self.dl_slots[c].reshape(self.T * self.CS, P).T
        return np.ascontiguousarray(dd).astype(BF)


# --------------------------------------------------------------------------
# device kernel builder (shared by both layers)
# --------------------------------------------------------------------------

def build_layer_kernel(T, CS, npc, layer, n_cores):
    """layer 1: MW=128, H=8, CSZ=16, out h bf16 (ELU'd)
       layer 2: MW=40,  H=1, CSZ=40, out logits f32"""
    if layer == 1:
        MW, H, CSZ = 128, 8, 16
        MSTRIDE = 136          # mea slot stride (elems)
        PSTRIDE = 256          # pa chunk stride (f32)
    else:
        MW, H, CSZ = 40, 1, 40
        MSTRIDE = 44
        PSTRIDE = 64
    WC = MW + H
    nslots = T * CS * P
    NQ = 4                                  # quarters per tile
    QC = math.ceil(CS / NQ)                 # chunks per quarter (last short)
    # quarter q covers global chunks [q*QC, min((q+1)*QC, CS)); the self slot
    # is appended to the last quarter.
    assert (QC + 1) * PSTRIDE * 4 <= 8192, "pa tile exceeds 4 banks"

    nc = bacc.Bacc("TRN2", target_bir_lowering=False, debug=False,
                   num_devices=n_cores)
    ap = {}
    ap["xdT"] = nc.dram_tensor("xdT", [P, nslots], BF16,
                               kind="ExternalInput").ap()
    ap["xddT"] = nc.dram_tensor("xddT", [P, nslots], BF16,
                                kind="ExternalInput").ap()
    ap["xTm"] = nc.dram_tensor("xTm", [P, T * P], BF16,
                               kind="ExternalInput").ap()
    ap["dlt"] = nc.dram_tensor("dlt", [P, T * CS], BF16,
                               kind="ExternalInput").ap()
    ap["wext"] = nc.dram_tensor("wext", [P, WC], BF16,
                                kind="ExternalInput").ap()
    ap["wdst"] = nc.dram_tensor("wdst", [P, H], BF16,
                                kind="ExternalInput").ap()
    ap["wself"] = nc.dram_tensor("wself", [P, WC], BF16,
                                 kind="ExternalInput").ap()
    ap["iota_bf"] = nc.dram_tensor("iota_bf", [P, P], BF16,
                                   kind="ExternalInput").ap()
    ap["bias_rep"] = nc.dram_tensor("bias_rep", [P, MW], F32,
                                    kind="ExternalInput").ap()
    if layer == 1:
        out_ap = nc.dram_tensor("hout", [T * P, MW], BF16,
                                kind="ExternalOutput").ap()
    else:
        out_ap = nc.dram_tensor("logits", [T * P, MW], F32,
                                kind="ExternalOutput").ap()

    with tile.TileContext(nc) as tc, ExitStack() as ctx:
        cpool = ctx.enter_context(tc.tile_pool(name="consts", bufs=1))
        wext = cpool.tile([P, WC], BF16)
        nc.sync.dma_start(wext[:], ap["wext"])
        wdst = cpool.tile([P, H], BF16)
        nc.sync.dma_start(wdst[:], ap["wdst"])
        wself = cpool.tile([P, WC], BF16)
        nc.sync.dma_start(wself[:], ap["wself"])
        iota_bf = cpool.tile([P, P], BF16)
        nc.sync.dma_start(iota_bf[:], ap["iota_bf"])
        bias_rep = cpool.tile([P, MW], F32)
        nc.sync.dma_start(bias_rep[:], ap["bias_rep"])
        dlt = cpool.tile([P, T * CS], BF16)
        nc.sync.dma_start(dlt[:], ap["dlt"])

        sb = ctx.enter_context(tc.tile_pool(name="sb", bufs=2))
        sbm = ctx.enter_context(tc.tile_pool(name="sbm", bufs=2))
        epi = ctx.enter_context(tc.tile_pool(name="epi", bufs=2))
        ppa = ctx.enter_context(tc.tile_pool(name="ppa", bufs=2,
                                             space="PSUM"))
        pacc = ctx.enter_context(tc.tile_pool(name="pacc", bufs=2,
                                              space="PSUM"))

        for t in range(T):
            xd = sb.tile([P, CS * P], BF16, tag="xd")
            nc.sync.dma_start(xd[:], ap["xdT"][:, t * CS * P:(t + 1) * CS * P])
            xdd = sb.tile([P, CS * P], BF16, tag="xdd")
            nc.sync.dma_start(xdd[:],
                              ap["xddT"][:, t * CS * P:(t + 1) * CS * P])
            xo = sb.tile([P, P], BF16, tag="xo")
            nc.sync.dma_start(xo[:], ap["xTm"][:, t * P:(t + 1) * P])

            # seT for all chunks of the tile in one op (on gpsimd: DVE is hot)
            seT = sb.tile([P, CS * P], BF16, tag="seT")
            se3 = seT[:].rearrange("p (c d) -> p c d", d=P)
            in0 = dlt[:, t * CS:(t + 1) * CS].rearrange(
                "p (c o) -> p c o", o=1).to_broadcast([P, CS, P])
            in1 = iota_bf[:].rearrange("p (o d) -> p o d", o=1).to_broadcast(
                [P, CS, P])
            nc.vector.tensor_tensor(out=se3, in0=in0, in1=in1,
                                    op=ALU.is_equal)

            mea = sbm.tile([P, (CS + 1) * MSTRIDE], BF16, tag="mea")
            alre = sbm.tile([P, (CS + 1) * H], F32, tag="alre")
            acc = pacc.tile([P, WC], F32, tag="acc")

            for q in range(NQ):
                k0 = q * QC
                k1 = min(k0 + QC, CS)
                nk = k1 - k0              # real chunks in this quarter
                has_self = (q == NQ - 1)
                pa = ppa.tile([P, (QC + 1) * PSTRIDE], F32, tag="pa")
                pa3 = pa[:].rearrange("p (c e) -> p c e", e=PSTRIDE)
                ns = nk + (1 if has_self else 0)   # slots incl self
                for j in range(ns):
                    kg = k0 + j
                    if has_self and j == nk:
                        lhs = xo[:]
                    else:
                        lhs = xd[:, kg * P:(kg + 1) * P]
                    # [xh | es] in one matmul; ed accumulates onto es cols
                    nc.tensor.matmul(
                        out=pa[:, j * PSTRIDE:j * PSTRIDE + WC],
                        lhsT=lhs,
                        rhs=wself[:] if (has_self and j == nk) else wext[:],
                        start=True, stop=has_self and j == nk,
                        skip_group_check=True)
                    if not (has_self and j == nk):
                        nc.tensor.matmul(
                            out=pa[:, j * PSTRIDE + MW:j * PSTRIDE + WC],
                            lhsT=xdd[:, kg * P:(kg + 1) * P], rhs=wdst[:],
                            start=False, stop=True, skip_group_check=True)
                # exp(leaky_relu(a)) == max(exp(a), exp(slope*a))
                mq0 = mea[:].rearrange("p (c e) -> p c e", e=MSTRIDE)
                ea_out = mq0[:, k0:k0 + ns, MW:WC]
                nc.scalar.activation(
                    out=ea_out, in_=pa3[:, 0:ns, MW:WC],
                    func=AF.Exp)
                nc.scalar.activation(
                    out=alre[:, k0 * H:(k0 + ns) * H].rearrange(
                        "p (c h) -> p c h", h=H),
                    in_=pa3[:, 0:ns, MW:WC],
                    func=AF.Exp, scale=float(NEG_SLOPE))
                nc.vector.tensor_tensor(
                    out=ea_out, in0=ea_out,
                    in1=alre[:, k0 * H:(k0 + ns) * H].rearrange(
                        "p (c h) -> p c h", h=H),
                    op=ALU.max)
                # msg = ea * xh  (one vector op per quarter)
                mq = mea[:].rearrange("p (c e) -> p c e", e=MSTRIDE)
                ea_in = mq[:, k0:k0 + ns, MW:WC].rearrange(
                    "p c (h o) -> p c h o", o=1).to_broadcast([P, ns, H, CSZ])
                pa3 = pa[:].rearrange("p (c e) -> p c e", e=PSTRIDE)
                xh_in = pa3[:, 0:ns, 0:MW].rearrange(
                    "p c (h z) -> p c h z", z=CSZ)
                msg_out = mq[:, k0:k0 + ns, 0:MW].rearrange(
                    "p c (h z) -> p c h z", z=CSZ)
                nc.vector.tensor_tensor(out=msg_out, in0=ea_in, in1=xh_in,
                                        op=ALU.mult)
                # segment-reduce the real chunks into the accumulator
                for j in range(nk):
                    kg = k0 + j
                    nc.tensor.matmul(
                        out=acc[:],
                        lhsT=seT[:, kg * P:(kg + 1) * P],
                        rhs=mea[:, kg * MSTRIDE:kg * MSTRIDE + WC],
                        start=(kg == 0), stop=(kg == CS - 1),
                        skip_group_check=True)

            # ---- epilogue: add self slot, softmax-divide, bias (+ ELU) ----
            uden = epi.tile([P, WC], F32, tag="uden")
            nc.vector.tensor_tensor(
                out=uden[:], in0=acc[:],
                in1=mea[:, CS * MSTRIDE:CS * MSTRIDE + WC], op=ALU.add)
            rec = epi.tile([P, H], F32, tag="rec")
            nc.vector.reciprocal(out=rec[:], in_=uden[:, MW:WC])
            o = epi.tile([P, MW], F32, tag="o")
            if H == 1:
                nc.vector.tensor_tensor(
                    out=o[:], in0=rec[:, 0:1].to_broadcast([P, MW]),
                    in1=uden[:, 0:MW], op=ALU.mult)
            else:
                nc.vector.tensor_tensor(
                    out=o[:].rearrange("p (h z) -> p h z", z=CSZ),
                    in0=rec[:].rearrange("p (h o) -> p h o", o=1)
                    .to_broadcast([P, H, CSZ]),
                    in1=uden[:, 0:MW].rearrange("p (h z) -> p h z", z=CSZ),
                    op=ALU.mult)
            nc.vector.tensor_tensor(out=o[:], in0=o[:], in1=bias_rep[:],
                                    op=ALU.add)
            if layer == 1:
                tmp = epi.tile([P, MW], F32, tag="tmp")
                nc.vector.tensor_scalar_min(out=tmp[:], in0=o[:], scalar1=0.0)
                nc.scalar.activation(out=tmp[:], in_=tmp[:], func=AF.Exp)
                nc.vector.tensor_scalar_max(out=o[:], in0=o[:], scalar1=0.0)
                hrow = epi.tile([P, MW], BF16, tag="hrow")
                nc.vector.tensor_tensor(out=o[:], in0=o[:], in1=tmp[:],
                                        op=ALU.add)
                nc.vector.tensor_scalar_add(out=hrow[:], in0=o[:],
                                            scalar1=-1.0)
                nc.sync.dma_start(out_ap[t * P:(t + 1) * P, :], hrow[:])
            else:
                nc.sync.dma_start(out_ap[t * P:(t + 1) * P, :], o[:])

    nc.compile()
    return nc


# --------------------------------------------------------------------------
# host orchestration
# --------------------------------------------------------------------------

def _blockdiag(att):
    H, C = att.shape
    out = np.zeros((H * C, H), np.float32)
    for h in range(H):
        out[h * C:(h + 1) * C, h] = att[h]
    return out


def _iota_bf():
    return np.tile(np.arange(P, dtype=np.float32).astype(BF)[None, :], (P, 1))


def _own_T(tbl_bf, lo, npc, T):
    """[N, C] bf16 -> [C, T*128] bf16 (own nodes, transposed, zero-padded)."""
    out = np.zeros((T * P, tbl_bf.shape[1]), BF)
    out[:npc] = tbl_bf[lo:lo + npc]
    return np.ascontiguousarray(out.T)


def run_gat(x, edge_index, W1, att_src1, att_dst1, b1, W2, att_src2, att_dst2,
            b2, N, n_cores):
    src = np.asarray(edge_index[0]).astype(np.int64)
    dst = np.asarray(edge_index[1]).astype(np.int64)
    el = EdgeLayout(src, dst, N, n_cores)
    T, CS, npc = el.T, el.CS, el.npc

    x_bf = np.asarray(x, np.float32).astype(BF)
    W1 = np.asarray(W1, np.float32)
    bd_s1 = _blockdiag(np.asarray(att_src1, np.float32))
    bd_d1 = _blockdiag(np.asarray(att_dst1, np.float32))
    w1ext = np.concatenate([W1, W1 @ bd_s1], axis=1).astype(BF)
    w1dst = (W1 @ bd_d1).astype(BF)
    w1self = np.concatenate([W1, W1 @ (bd_s1 + bd_d1)], axis=1).astype(BF)
    bias1 = np.tile(np.asarray(b1, np.float32)[None, :], (P, 1))
    iota = _iota_bf()

    nc1 = build_layer_kernel(T, CS, npc, 1, n_cores)
    in_maps = []
    for c in range(n_cores):
        in_maps.append({
            "xdT": el.dup_T(x_bf, el.src_slots[c]),
            "xddT": el.dup_T(x_bf, el.dst_slots[c]),
            "xTm": _own_T(x_bf, c * npc, npc, T),
            "dlt": el.dlt(c),
            "wext": w1ext, "wdst": w1dst, "wself": w1self,
            "iota_bf": iota, "bias_rep": bias1,
        })
    res1 = run_bass_kernel_spmd(nc1, in_maps, core_ids=list(range(n_cores)))
    h_bf = np.zeros((N, P), BF)
    for c in range(n_cores):
        h_bf[c * npc:(c + 1) * npc] = res1.results[c]["hout"][:npc]

    W2 = np.asarray(W2, np.float32)
    bd_s2 = _blockdiag(np.asarray(att_src2, np.float32))
    bd_d2 = _blockdiag(np.asarray(att_dst2, np.float32))
    w2ext = np.concatenate([W2, W2 @ bd_s2], axis=1).astype(BF)
    w2dst = (W2 @ bd_d2).astype(BF)
    w2self = np.concatenate([W2, W2 @ (bd_s2 + bd_d2)], axis=1).astype(BF)
    bias2 = np.tile(np.asarray(b2, np.float32)[None, :], (P, 1))

    nc2 = build_layer_kernel(T, CS, npc, 2, n_cores)
    in_maps2 = []
    for c in range(n_cores):
        in_maps2.append({
            "xdT": el.dup_T(h_bf, el.src_slots[c]),
            "xddT": el.dup_T(h_bf, el.dst_slots[c]),
            "xTm": _own_T(h_bf, c * npc, npc, T),
            "dlt": el.dlt(c),
            "wext": w2ext, "wdst": w2dst, "wself": w2self,
            "iota_bf": iota, "bias_rep": bias2,
        })
    res2 = run_bass_kernel_spmd(nc2, in_maps2, core_ids=list(range(n_cores)))
    out = np.zeros((N, 40), np.float32)
    for c in range(n_cores):
        out[c * npc:(c + 1) * npc] = res2.results[c]["logits"][:npc, :40]
    return out


def kernel(x, edge_index, W1, att_src1, att_dst1, b1, W2, att_src2, att_dst2,
           b2):
    N = int(np.asarray(x).shape[0])
    return run_gat(x, edge_index, W1, att_src1, att_dst1, b1, W2, att_src2,
                   att_dst2, b2, N=N, n_cores=8)


# revision 14
# speedup vs baseline: 1.0870x; 1.0870x over previous
"""GAT (2-layer, PyG-style) distributed Bass kernel for 8 Trainium2 NeuronCores.

Strategy (graph/data parallel; halo exchange done at input-sharding time):
  - Nodes are partitioned into 8 contiguous blocks; core c owns destination
    nodes [c*N/8, (c+1)*N/8) and all edges incident to them.
  - The host shards the inputs per core as edge-ordered, transposed bf16
    feature arrays: xdT[:, i] = x[src_i], xddT[:, i] = x[dst_i] for each
    edge slot i (dst-tile-major layout, 128-slot chunks).  This is the halo
    exchange of neighbor features performed eagerly during input
    distribution, so the device sees only contiguous DMA.
  - Device per dst tile (128 nodes, CS chunks of 128 edges):
      * per chunk: matmul (xdT_chunk)^T @ [W|W@a_src] -> [xh|es] in PSUM;
        es+ed accumulated in one PSUM bank via two matmuls
        (lhsT=xdT chunk then lhsT=xddT chunk);
      * batched leaky-relu + exp on the scalar engine per quarter-tile;
      * one vector multiply per quarter forms [att*xh | ea] rows (bf16);
      * per chunk one bf16 selection matmul (seT built by a single batched
        is_equal per tile) segment-reduces [msg|ea] into the accumulator;
      * self loops are handled as an extra slot per tile (identity
        selection -> added in the epilogue, no matmul).
  - Between layers the host reassembles h and builds the layer-2 dup arrays
    the same way.
"""

import math
import os
import sys

for _p in ("/opt/trn_rl_repo", "/root/.axon_site/_ro/trn_rl_repo"):
    if os.path.isdir(_p) and _p not in sys.path:
        sys.path.insert(0, _p)

import numpy as np
import ml_dtypes
from contextlib import ExitStack

import concourse.bacc as bacc
import concourse.bass as bass
import concourse.tile as tile
from concourse import mybir
from concourse.bass_utils import run_bass_kernel_spmd

F32 = mybir.dt.float32
BF16 = mybir.dt.bfloat16
AF = mybir.ActivationFunctionType
ALU = mybir.AluOpType
BF = ml_dtypes.bfloat16

NEG_SLOPE = 0.2
P = 128
PAD_DST = 200.0  # sentinel dst_local for pad/self slots


def _round_up(a, b):
    return (a + b - 1) // b * b


# --------------------------------------------------------------------------
# host-side graph preprocessing (pure indexing, no float math on features)
# --------------------------------------------------------------------------

class EdgeLayout:
    def __init__(self, src, dst, N, n_cores):
        self.N = N
        self.n_cores = n_cores
        assert N % n_cores == 0
        self.npc = N // n_cores
        self.T = math.ceil(self.npc / P)

        src = src.astype(np.int64)
        dst = dst.astype(np.int64)

        per_core = []
        max_cnt = 0
        for c in range(n_cores):
            lo = c * self.npc
            sel = (dst >= lo) & (dst < lo + self.npc)
            s_c = src[sel]
            d_c = dst[sel]
            dl = d_c - lo
            t_all = dl >> 7
            order = np.argsort(t_all, kind="stable")
            s_c, d_c, dl, t_all = s_c[order], d_c[order], dl[order], t_all[order]
            cnt = np.bincount(t_all, minlength=self.T)
            max_cnt = max(max_cnt, int(cnt.max()))
            per_core.append((s_c, d_c, dl, cnt))

        self.CS = max(1, math.ceil(max_cnt / P))   # chunks per tile
        CS = self.CS
        self.nslots = self.T * CS * P

        self.src_slots = []   # int64 [nslots], -1 for pad
        self.dst_slots = []
        self.dl_slots = []    # float32 [nslots], PAD_DST for pad
        for c in range(n_cores):
            s_c, d_c, dl, cnt = per_core[c]
            ss = np.full(self.nslots, -1, np.int64)
            ds = np.full(self.nslots, -1, np.int64)
            dd = np.full(self.nslots, PAD_DST, np.float32)
            starts = np.concatenate([[0], np.cumsum(cnt)])[:-1]
            pos_in_tile = np.arange(len(s_c)) - starts[dl >> 7]
            pos = (dl >> 7) * (CS * P) + pos_in_tile
            ss[pos] = s_c
            ds[pos] = d_c
            dd[pos] = (dl & 127).astype(np.float32)
            self.src_slots.append(ss)
            self.dst_slots.append(ds)
            self.dl_slots.append(dd)

    def dup_T(self, tbl_bf, slots):
        """tbl_bf: [N, C] bf16 -> [C, nslots] bf16 (zeros at pad slots)."""
        out = np.zeros((self.nslots, tbl_bf.shape[1]), BF)
        real = slots >= 0
        out[real] = tbl_bf[slots[real]]
        return np.ascontiguousarray(out.T)

    def dlt(self, c):
        """[128, T*CS] bf16: dl of slot (t, k, p) at [p, t*CS+k]."""
        dd = self.dl_slots[c].reshape(self.T * self.CS, P).T
        return np.ascontiguousarray(dd).astype(BF)


# --------------------------------------------------------------------------
# device kernel builder (shared by both layers)
# --------------------------------------------------------------------------

def build_layer_kernel(T, CS, npc, layer, n_cores):
    """layer 1: MW=128, H=8, CSZ=16, out h bf16 (ELU'd)
       layer 2: MW=40,  H=1, CSZ=40, out logits f32"""
    if layer == 1:
        MW, H, CSZ = 128, 8, 16
        MSTRIDE = 136          # mea slot stride (elems)
        PSTRIDE = 256          # pa chunk stride (f32)
    else:
        MW, H, CSZ = 40, 1, 40
        MSTRIDE = 44
        PSTRIDE = 64
    WC = MW + H
    nslots = T * CS * P
    NQ = 4                                  # quarters per tile
    QC = math.ceil(CS / NQ)                 # chunks per quarter (last short)
    # quarter q covers global chunks [q*QC, min((q+1)*QC, CS)); the self slot
    # is appended to the last quarter.
    assert (QC + 1) * PSTRIDE * 4 <= 8192, "pa tile exceeds 4 banks"

    nc = bacc.Bacc("TRN2", target_bir_lowering=False, debug=False,
                   num_devices=n_cores)
    ap = {}
    ap["xdT"] = nc.dram_tensor("xdT", [P, nslots], BF16,
                               kind="ExternalInput").ap()
    ap["xddT"] = nc.dram_tensor("xddT", [P, nslots], BF16,
                                kind="ExternalInput").ap()
    ap["xTm"] = nc.dram_tensor("xTm", [P, T * P], BF16,
                               kind="ExternalInput").ap()
    ap["dlt"] = nc.dram_tensor("dlt", [P, T * CS], BF16,
                               kind="ExternalInput").ap()
    ap["wext"] = nc.dram_tensor("wext", [P, WC], BF16,
                                kind="ExternalInput").ap()
    ap["wdst"] = nc.dram_tensor("wdst", [P, H], BF16,
                                kind="ExternalInput").ap()
    ap["wself"] = nc.dram_tensor("wself", [P, WC], BF16,
                                 kind="ExternalInput").ap()
    ap["iota_bf"] = nc.dram_tensor("iota_bf", [P, P], BF16,
                                   kind="ExternalInput").ap()
    ap["bias_rep"] = nc.dram_tensor("bias_rep", [P, MW], F32,
                                    kind="ExternalInput").ap()
    if layer == 1:
        out_ap = nc.dram_tensor("hout", [T * P, MW], BF16,
                                kind="ExternalOutput").ap()
    else:
        out_ap = nc.dram_tensor("logits", [T * P, MW], F32,
                                kind="ExternalOutput").ap()

    with tile.TileContext(nc) as tc, ExitStack() as ctx:
        cpool = ctx.enter_context(tc.tile_pool(name="consts", bufs=1))
        wext = cpool.tile([P, WC], BF16)
        nc.sync.dma_start(wext[:], ap["wext"])
        wdst = cpool.tile([P, H], BF16)
        nc.sync.dma_start(wdst[:], ap["wdst"])
        wself = cpool.tile([P, WC], BF16)
        nc.sync.dma_start(wself[:], ap["wself"])
        iota_bf = cpool.tile([P, P], BF16)
        nc.sync.dma_start(iota_bf[:], ap["iota_bf"])
        bias_rep = cpool.tile([P, MW], F32)
        nc.sync.dma_start(bias_rep[:], ap["bias_rep"])
        dlt = cpool.tile([P, T * CS], BF16)
        nc.sync.dma_start(dlt[:], ap["dlt"])

        sb = ctx.enter_context(tc.tile_pool(name="sb", bufs=3))
        sbm = ctx.enter_context(tc.tile_pool(name="sbm", bufs=3))
        epi = ctx.enter_context(tc.tile_pool(name="epi", bufs=3))
        ppa = ctx.enter_context(tc.tile_pool(name="ppa", bufs=2,
                                             space="PSUM"))
        pacc = ctx.enter_context(tc.tile_pool(name="pacc", bufs=2,
                                              space="PSUM"))

        for t in range(T):
            xd = sb.tile([P, CS * P], BF16, tag="xd")
            nc.sync.dma_start(xd[:], ap["xdT"][:, t * CS * P:(t + 1) * CS * P])
            xdd = sb.tile([P, CS * P], BF16, tag="xdd")
            nc.sync.dma_start(xdd[:],
                              ap["xddT"][:, t * CS * P:(t + 1) * CS * P])
            xo = sb.tile([P, P], BF16, tag="xo")
            nc.sync.dma_start(xo[:], ap["xTm"][:, t * P:(t + 1) * P])

            # seT for all chunks of the tile in one op (on gpsimd: DVE is hot)
            seT = sb.tile([P, CS * P], BF16, tag="seT")
            se3 = seT[:].rearrange("p (c d) -> p c d", d=P)
            in0 = dlt[:, t * CS:(t + 1) * CS].rearrange(
                "p (c o) -> p c o", o=1).to_broadcast([P, CS, P])
            in1 = iota_bf[:].rearrange("p (o d) -> p o d", o=1).to_broadcast(
                [P, CS, P])
            nc.vector.tensor_tensor(out=se3, in0=in0, in1=in1,
                                    op=ALU.is_equal)

            mea = sbm.tile([P, (CS + 1) * MSTRIDE], BF16, tag="mea")
            alre = sbm.tile([P, (CS + 1) * H], F32, tag="alre")
            acc = pacc.tile([P, WC], F32, tag="acc")

            for q in range(NQ):
                k0 = q * QC
                k1 = min(k0 + QC, CS)
                nk = k1 - k0              # real chunks in this quarter
                has_self = (q == NQ - 1)
                pa = ppa.tile([P, (QC + 1) * PSTRIDE], F32, tag="pa")
                pa3 = pa[:].rearrange("p (c e) -> p c e", e=PSTRIDE)
                ns = nk + (1 if has_self else 0)   # slots incl self
                for j in range(ns):
                    kg = k0 + j
                    if has_self and j == nk:
                        lhs = xo[:]
                    else:
                        lhs = xd[:, kg * P:(kg + 1) * P]
                    # [xh | es] in one matmul; ed accumulates onto es cols
                    nc.tensor.matmul(
                        out=pa[:, j * PSTRIDE:j * PSTRIDE + WC],
                        lhsT=lhs,
                        rhs=wself[:] if (has_self and j == nk) else wext[:],
                        start=True, stop=has_self and j == nk,
                        skip_group_check=True)
                    if not (has_self and j == nk):
                        nc.tensor.matmul(
                            out=pa[:, j * PSTRIDE + MW:j * PSTRIDE + WC],
                            lhsT=xdd[:, kg * P:(kg + 1) * P], rhs=wdst[:],
                            start=False, stop=True, skip_group_check=True)
                # exp(leaky_relu(a)) == max(exp(a), exp(slope*a))
                mq0 = mea[:].rearrange("p (c e) -> p c e", e=MSTRIDE)
                ea_out = mq0[:, k0:k0 + ns, MW:WC]
                nc.scalar.activation(
                    out=ea_out, in_=pa3[:, 0:ns, MW:WC],
                    func=AF.Exp)
                nc.scalar.activation(
                    out=alre[:, k0 * H:(k0 + ns) * H].rearrange(
                        "p (c h) -> p c h", h=H),
                    in_=pa3[:, 0:ns, MW:WC],
                    func=AF.Exp, scale=float(NEG_SLOPE))
                nc.vector.tensor_tensor(
                    out=ea_out, in0=ea_out,
                    in1=alre[:, k0 * H:(k0 + ns) * H].rearrange(
                        "p (c h) -> p c h", h=H),
                    op=ALU.max)
                # msg = ea * xh  (one vector op per quarter)
                mq = mea[:].rearrange("p (c e) -> p c e", e=MSTRIDE)
                ea_in = mq[:, k0:k0 + ns, MW:WC].rearrange(
                    "p c (h o) -> p c h o", o=1).to_broadcast([P, ns, H, CSZ])
                pa3 = pa[:].rearrange("p (c e) -> p c e", e=PSTRIDE)
                xh_in = pa3[:, 0:ns, 0:MW].rearrange(
                    "p c (h z) -> p c h z", z=CSZ)
                msg_out = mq[:, k0:k0 + ns, 0:MW].rearrange(
                    "p c (h z) -> p c h z", z=CSZ)
                nc.vector.tensor_tensor(out=msg_out, in0=ea_in, in1=xh_in,
                                        op=ALU.mult)
            # segment-reduce all chunks after the quarter pipeline so the
            # in-order PE queue never stalls behind the vector multiplies
            for kg in range(CS):
                nc.tensor.matmul(
                    out=acc[:],
                    lhsT=seT[:, kg * P:(kg + 1) * P],
                    rhs=mea[:, kg * MSTRIDE:kg * MSTRIDE + WC],
                    start=(kg == 0), stop=(kg == CS - 1),
                    skip_group_check=True)

            # ---- epilogue: add self slot, softmax-divide, bias (+ ELU) ----
            uden = epi.tile([P, WC], F32, tag="uden")
            nc.vector.tensor_tensor(
                out=uden[:], in0=acc[:],
                in1=mea[:, CS * MSTRIDE:CS * MSTRIDE + WC], op=ALU.add)
            rec = epi.tile([P, H], F32, tag="rec")
            nc.vector.reciprocal(out=rec[:], in_=uden[:, MW:WC])
            o = epi.tile([P, MW], F32, tag="o")
            if H == 1:
                nc.vector.tensor_tensor(
                    out=o[:], in0=rec[:, 0:1].to_broadcast([P, MW]),
                    in1=uden[:, 0:MW], op=ALU.mult)
            else:
                nc.vector.tensor_tensor(
                    out=o[:].rearrange("p (h z) -> p h z", z=CSZ),
                    in0=rec[:].rearrange("p (h o) -> p h o", o=1)
                    .to_broadcast([P, H, CSZ]),
                    in1=uden[:, 0:MW].rearrange("p (h z) -> p h z", z=CSZ),
                    op=ALU.mult)
            nc.vector.tensor_tensor(out=o[:], in0=o[:], in1=bias_rep[:],
                                    op=ALU.add)
            if layer == 1:
                tmp = epi.tile([P, MW], F32, tag="tmp")
                nc.vector.tensor_scalar_min(out=tmp[:], in0=o[:], scalar1=0.0)
                nc.scalar.activation(out=tmp[:], in_=tmp[:], func=AF.Exp)
                nc.vector.tensor_scalar_max(out=o[:], in0=o[:], scalar1=0.0)
                hrow = epi.tile([P, MW], BF16, tag="hrow")
                nc.vector.tensor_tensor(out=o[:], in0=o[:], in1=tmp[:],
                                        op=ALU.add)
                nc.vector.tensor_scalar_add(out=hrow[:], in0=o[:],
                                            scalar1=-1.0)
                nc.sync.dma_start(out_ap[t * P:(t + 1) * P, :], hrow[:])
            else:
                nc.sync.dma_start(out_ap[t * P:(t + 1) * P, :], o[:])

    nc.compile()
    return nc


# --------------------------------------------------------------------------
# host orchestration
# --------------------------------------------------------------------------

def _blockdiag(att):
    H, C = att.shape
    out = np.zeros((H * C, H), np.float32)
    for h in range(H):
        out[h * C:(h + 1) * C, h] = att[h]
    return out


def _iota_bf():
    return np.tile(np.arange(P, dtype=np.float32).astype(BF)[None, :], (P, 1))


def _own_T(tbl_bf, lo, npc, T):
    """[N, C] bf16 -> [C, T*128] bf16 (own nodes, transposed, zero-padded)."""
    out = np.zeros((T * P, tbl_bf.shape[1]), BF)
    out[:npc] = tbl_bf[lo:lo + npc]
    return np.ascontiguousarray(out.T)


def run_gat(x, edge_index, W1, att_src1, att_dst1, b1, W2, att_src2, att_dst2,
            b2, N, n_cores):
    src = np.asarray(edge_index[0]).astype(np.int64)
    dst = np.asarray(edge_index[1]).astype(np.int64)
    el = EdgeLayout(src, dst, N, n_cores)
    T, CS, npc = el.T, el.CS, el.npc

    x_bf = np.asarray(x, np.float32).astype(BF)
    W1 = np.asarray(W1, np.float32)
    bd_s1 = _blockdiag(np.asarray(att_src1, np.float32))
    bd_d1 = _blockdiag(np.asarray(att_dst1, np.float32))
    w1ext = np.concatenate([W1, W1 @ bd_s1], axis=1).astype(BF)
    w1dst = (W1 @ bd_d1).astype(BF)
    w1self = np.concatenate([W1, W1 @ (bd_s1 + bd_d1)], axis=1).astype(BF)
    bias1 = np.tile(np.asarray(b1, np.float32)[None, :], (P, 1))
    iota = _iota_bf()

    nc1 = build_layer_kernel(T, CS, npc, 1, n_cores)
    in_maps = []
    for c in range(n_cores):
        in_maps.append({
            "xdT": el.dup_T(x_bf, el.src_slots[c]),
            "xddT": el.dup_T(x_bf, el.dst_slots[c]),
            "xTm": _own_T(x_bf, c * npc, npc, T),
            "dlt": el.dlt(c),
            "wext": w1ext, "wdst": w1dst, "wself": w1self,
            "iota_bf": iota, "bias_rep": bias1,
        })
    res1 = run_bass_kernel_spmd(nc1, in_maps, core_ids=list(range(n_cores)))
    h_bf = np.zeros((N, P), BF)
    for c in range(n_cores):
        h_bf[c * npc:(c + 1) * npc] = res1.results[c]["hout"][:npc]

    W2 = np.asarray(W2, np.float32)
    bd_s2 = _blockdiag(np.asarray(att_src2, np.float32))
    bd_d2 = _blockdiag(np.asarray(att_dst2, np.float32))
    w2ext = np.concatenate([W2, W2 @ bd_s2], axis=1).astype(BF)
    w2dst = (W2 @ bd_d2).astype(BF)
    w2self = np.concatenate([W2, W2 @ (bd_s2 + bd_d2)], axis=1).astype(BF)
    bias2 = np.tile(np.asarray(b2, np.float32)[None, :], (P, 1))

    nc2 = build_layer_kernel(T, CS, npc, 2, n_cores)
    in_maps2 = []
    for c in range(n_cores):
        in_maps2.append({
            "xdT": el.dup_T(h_bf, el.src_slots[c]),
            "xddT": el.dup_T(h_bf, el.dst_slots[c]),
            "xTm": _own_T(h_bf, c * npc, npc, T),
            "dlt": el.dlt(c),
            "wext": w2ext, "wdst": w2dst, "wself": w2self,
            "iota_bf": iota, "bias_rep": bias2,
        })
    res2 = run_bass_kernel_spmd(nc2, in_maps2, core_ids=list(range(n_cores)))
    out = np.zeros((N, 40), np.float32)
    for c in range(n_cores):
        out[c * npc:(c + 1) * npc] = res2.results[c]["logits"][:npc, :40]
    return out


def kernel(x, edge_index, W1, att_src1, att_dst1, b1, W2, att_src2, att_dst2,
           b2):
    N = int(np.asarray(x).shape[0])
    return run_gat(x, edge_index, W1, att_src1, att_dst1, b1, W2, att_src2,
                   att_dst2, b2, N=N, n_cores=8)
